# revision 11
# baseline (speedup 1.0000x reference)
"""DiffGraphTransformer attention kernel for 8x Trainium2 NeuronCores.

Reference computation (T=1024, B=8, E=512, H=8, hd=64):
    qkv = query @ in_proj_weight.T + in_proj_bias ; q,k,v = split(qkv)
    k = q ; q *= hd**-0.5
    per (batch,head): scores = q @ k.T            (T,T)
                      w = exp(scores - max) * pe[b]
                      w /= clip(sum(w,-1), 1e-6)
                      attn = w @ v
    out = attn @ out_proj_weight.T + out_proj_bias

Sharding: batch b -> core b.  Heads 8b..8b+7 all use pe[b], so each core is
fully independent (pure SPMD, no collectives, full inputs sharded on host).

Design (v2, ACT-paced):
  * k == q, so the k-chunk of in_proj is dead weight; only Wq / Wv used.
  * softmax max-subtraction replaced by a constant shift (exp(s/8 - 10)).
  * S = q q^T symmetric: E = exp(S) stored [s, t] times pe^T gives the
    contraction-major attention operand directly, no (T,T) transpose.
  * attention lhsT = [v_h | ones] (128, 65): PSUM row 64 = softmax denom.
  * ACT (exp) is the pacing engine: 64 x (128,1024) exps ~ 1.15us each =
    ~73us floor.  Everything else is held strictly below that:
      - whole operand path in fp16 (xT/wq/wv/qT/peT/E/W/v/attnT/wo):
        PE cycle count unchanged, DMA ramp halved, DVE muls in 2x mode.
      - normalization FUSED into the PSUM evacuation (attnT = psum * rm,
        one DVE op instead of copy+mul), reciprocal computed directly on
        the PSUM denominator rows, broadcast via a DRAM bounce.
      - qT (m>=1) evacuations on GPSIMD; 2 of 16 pe-mults per pair on
        GPSIMD; the rest of the elementwise work on DVE (~16us/pair
        against an 18.4us pair period).
  * HAM: a warm-up matmul spam during the initial DMA window plus a dense
    PE stream (scores + attention interleave) keeps K=8/8 the whole body.
  * Biases are applied (bq in the qT evac, v-bias folded into bo2 which is
    added via a K=1 ones matmul) even though setup_inputs zeroes them.
"""

import sys

for _p in ("/opt/trn_rl_repo",):
    if _p not in sys.path:
        sys.path.insert(0, _p)

import numpy as np

T, B, E = 1024, 8, 512
H = 8
HD = E // H  # 64
N_CORES = 8

# global constant subtracted inside exp (cancels in normalization; keeps
# exp() outputs inside fp16 range: scores/8 - 10 is in [-16, ~6])
EXP_SHIFT = -10.0

_cache = {}


def _build_nc():
    import concourse.bass as bass
    import concourse.tile as tile
    import concourse.mybir as mybir
    from concourse import bacc
    from contextlib import ExitStack

    f32 = mybir.dt.float32
    fp16 = mybir.dt.float16
    Exp = mybir.ActivationFunctionType.Exp

    nc = bacc.Bacc("TRN2", debug=False)

    # DRAM I/O (per-core contents supplied via in_maps)
    xT_d = nc.dram_tensor("xT", [E, T], fp16, kind="ExternalInput").ap()
    peT_d = nc.dram_tensor("peT", [T, T], fp16, kind="ExternalInput").ap()
    wqT_d = nc.dram_tensor("wqT", [E, E], fp16, kind="ExternalInput").ap()
    wvT_d = nc.dram_tensor("wvT", [E, E], fp16, kind="ExternalInput").ap()
    woT_d = nc.dram_tensor("woT", [E, E], fp16, kind="ExternalInput").ap()
    bq_d = nc.dram_tensor("bq", [E], f32, kind="ExternalInput").ap()
    bo2_d = nc.dram_tensor("bo2", [E], fp16, kind="ExternalInput").ap()
    ones_d = nc.dram_tensor("ones1", [128], fp16, kind="ExternalInput").ap()
    out_d = nc.dram_tensor("out", [T, E], f32, kind="ExternalOutput").ap()

    KT = E // 128   # 4 contraction tiles for the projections
    TT = T // 128   # 8 t-tiles
    NH = T // 512   # 2 psum-bank halves of the t dimension
    NP = H // 2     # head pairs

    with ExitStack() as ctx:
        tc = ctx.enter_context(tile.TileContext(nc))

        sing = ctx.enter_context(tc.tile_pool(name="sing", bufs=1))
        p_in = ctx.enter_context(tc.tile_pool(name="p_in", bufs=1))
        p_qv = ctx.enter_context(tc.tile_pool(name="p_qv", bufs=1))
        p_E = ctx.enter_context(tc.tile_pool(name="p_E", bufs=8))
        p_W = ctx.enter_context(tc.tile_pool(name="p_W", bufs=20))
        p_rr = ctx.enter_context(tc.tile_pool(name="p_rr", bufs=2))
        p_rm = ctx.enter_context(tc.tile_pool(name="p_rm", bufs=4))
        p_st = ctx.enter_context(tc.tile_pool(name="p_st", bufs=2))
        p_dr = ctx.enter_context(tc.tile_pool(name="p_dr", bufs=2, space="DRAM"))
        ps_a = ctx.enter_context(tc.tile_pool(name="ps_a", bufs=2, space="PSUM"))
        ps_b = ctx.enter_context(tc.tile_pool(name="ps_b", bufs=4, space="PSUM"))

        # ---- t=0: ACT table preload + PE warm-up spam ---------------------
        # The exp table load (~2.7us) would otherwise serialize in front of
        # the first real exp; the warm-up matmuls keep the HAM activity
        # window busy during the initial DMA so the PE enters the first
        # projection already at K=8/8.
        dmy = sing.tile([1, 16], f32, tag="dmy")
        nc.vector.memset(dmy, 0.0)
        dmy2 = sing.tile([1, 16], f32, tag="dmy2")
        nc.scalar.activation(out=dmy2, in_=dmy, func=Exp, scale=1.0, bias=0.0)
        warm = sing.tile([128, 512], fp16, tag="warm")
        nc.vector.memset(warm, 0.0)
        wps = ps_b.tile([128, 512], f32, tag="slot", name="wps")
        for _ in range(10):
            nc.tensor.matmul(wps, warm[:, 0:128], warm, start=True, stop=True)

        # ---- constants / weights into SBUF --------------------------------
        # DMA order matters: first projection needs xT + wq m=0 columns only.
        wq_sb = [sing.tile([128, E], fp16, tag=f"wq{k}", name="wq") for k in range(KT)]
        wv_sb = [sing.tile([128, E], fp16, tag=f"wv{k}", name="wv") for k in range(KT)]
        wo_sb = [sing.tile([128, E], fp16, tag=f"wo{k}", name="wo") for k in range(KT)]
        bq_sb = [sing.tile([128, 1], f32, tag=f"bq{k}", name="bq") for k in range(KT)]
        xT_sb = [p_in.tile([128, T], fp16, tag=f"xT{k}", name="xT") for k in range(KT)]
        peT_sb = [p_in.tile([128, T], fp16, tag=f"peT{i}", name="peT") for i in range(TT)]
        for k in range(KT):
            nc.sync.dma_start(out=xT_sb[k], in_=xT_d[k * 128:(k + 1) * 128, :])
            nc.sync.dma_start(out=wq_sb[k][:, 0:128], in_=wqT_d[k * 128:(k + 1) * 128, 0:128])
            nc.sync.dma_start(out=bq_sb[k], in_=bq_d[k * 128:(k + 1) * 128].rearrange("(p one) -> p one", one=1))
        for k in range(KT):
            nc.sync.dma_start(out=wq_sb[k][:, 128:E], in_=wqT_d[k * 128:(k + 1) * 128, 128:E])
        ebias = sing.tile([128, 1], f32, tag="ebias")
        nc.vector.memset(ebias, EXP_SHIFT)
        nc.sync.dma_start(out=peT_sb[0], in_=peT_d[0:128, :])
        for k in range(KT):
            nc.sync.dma_start(out=wv_sb[k], in_=wvT_d[k * 128:(k + 1) * 128, :])
        for i in range(1, TT):
            nc.sync.dma_start(out=peT_sb[i], in_=peT_d[i * 128:(i + 1) * 128, :])
        ones1 = sing.tile([1, 128], fp16, tag="ones1")
        nc.sync.dma_start(out=ones1, in_=ones_d.unsqueeze(0))
        bo2_sb = sing.tile([1, E], fp16, tag="bo2")
        nc.sync.dma_start(out=bo2_sb, in_=bo2_d.unsqueeze(0))
        for k in range(KT):
            nc.sync.dma_start(out=wo_sb[k], in_=woT_d[k * 128:(k + 1) * 128, :])

        # ---- P1: projections ----------------------------------------------
        qT_sb = [p_qv.tile([128, T], fp16, tag=f"qT{k}", name="qT") for k in range(KT)]
        # v[t, e_out] natural, with a ones column appended per head:
        # layout (128, 8*65): head h occupies cols [65h, 65h+64), ones at 65h+64.
        v_sb = [p_qv.tile([128, H * (HD + 1)], fp16, tag=f"v{k}", name="v") for k in range(TT)]

        def emit_qproj(m, nh):
            ps = ps_b.tile([128, 512], f32, tag="slot", name="pp")
            for k in range(KT):
                nc.tensor.matmul(
                    ps, wq_sb[k][:, m * 128:(m + 1) * 128],
                    xT_sb[k][:, nh * 512:(nh + 1) * 512],
                    start=(k == 0), stop=(k == KT - 1))
            nc.vector.tensor_scalar_add(
                qT_sb[m][:, nh * 512:(nh + 1) * 512], ps, bq_sb[m])

        def emit_vproj(mt):
            ps = ps_b.tile([128, 512], f32, tag="slot", name="pp")
            for k in range(KT):
                nc.tensor.matmul(
                    ps, xT_sb[k][:, mt * 128:(mt + 1) * 128], wv_sb[k],
                    start=(k == 0), stop=(k == KT - 1))
            v_dst = v_sb[mt].rearrange("p (h c) -> p h c", c=HD + 1)
            nc.vector.tensor_copy(
                v_dst[:, :, 0:HD],
                ps.rearrange("p (h c) -> p h c", c=HD))
            nc.vector.memset(v_dst[:, :, HD:HD + 1], 1.0)

        # pair 0's qT upfront (fast DVE evac: it gates the first scores);
        # the other 14 projection groups interleave with iteration 0.
        for nh in range(NH):
            emit_qproj(0, nh)
        proj_rest = [("q", m, nh) for m in range(1, KT) for nh in range(NH)]
        proj_rest += [("v", mt, None) for mt in range(TT)]

        # ---- P2: attention, software-pipelined over head pairs ------------
        attnT_sb = [p_qv.tile([128, T], fp16, tag=f"attnT{k}", name="attnT") for k in range(KT)]
        Ws_of = {}   # pair j -> [hh][i] W' tiles
        at_of = {}   # pair j -> [hh][nh] psum accumulators

        def emit_scores(j, i):
            # nh-major emission: consecutive MMs alternate row groups
            # (head A rows 0-63, head B rows 64-127) so the PE runs the
            # pair concurrently and overlaps B's LDWEIGHTS with A's MM.
            qt = qT_sb[j]
            scs = [ps_a.tile([128, T], f32, tag="slot", name="sc") for _ in range(2)]
            # hh-major emission: h0's tile is freed by exp(i-1,h0) one ACT op
            # earlier than h1's, so h0's MMs run during exp(i-1,h1) and
            # exp(i,h0) starts with zero gap -- the ACT stream stays dense.
            for hh in range(2):
                r0 = hh * HD
                for nh in range(NH):
                    nc.tensor.matmul(
                        scs[hh][:, nh * 512:(nh + 1) * 512],
                        qt[r0:r0 + HD, i * 128:(i + 1) * 128],
                        qt[r0:r0 + HD, nh * 512:(nh + 1) * 512],
                        start=True, stop=True,
                        tile_position=(r0, 0))
            for hh in range(2):
                Et = p_E.tile([128, T], fp16, tag="E", name="Et")
                nc.scalar.activation(out=Et, in_=scs[hh], func=Exp, scale=0.125, bias=ebias)
                Wt = p_W.tile([128, T], fp16, tag="W", name="Wt")
                # GPSIMD relief: iteration 0 sends one mult per i to GPSIMD
                # (DVE is saturated with the projection evacuations there);
                # steady state sends two per pair.
                if hh == 0 and (j == 0 or i in (5, 6)):
                    nc.gpsimd.tensor_mul(Wt, Et, peT_sb[i])
                else:
                    nc.vector.tensor_mul(Wt, Et, peT_sb[i])
                Ws_of[j][hh][i] = Wt

        def alloc_at(j):
            at_of[j] = [[ps_b.tile([HD + 1, 512], f32, tag="slot", name="at")
                         for _ in range(NH)] for _ in range(2)]

        def emit_attn_kstep(j, i):
            for hh in range(2):
                vcol = 65 * (2 * j + hh)
                for nh in range(NH):
                    nc.tensor.matmul(
                        at_of[j][hh][nh],
                        v_sb[i][:, vcol:vcol + HD + 1],
                        Ws_of[j][hh][i][:, nh * 512:(nh + 1) * 512],
                        start=(i == 0), stop=(i == TT - 1))

        def emit_recip(j):
            # reciprocal of the pair's 4 denominator rows, straight off
            # PSUM, into one SBUF row; bounce through DRAM for the
            # partition-broadcast the normalization needs.
            rr = p_rr.tile([1, 4 * 512], f32, tag="rr", name="rr")
            for hh in range(2):
                for nh in range(NH):
                    r = hh * NH + nh
                    nc.vector.reciprocal(
                        rr[0:1, r * 512:(r + 1) * 512],
                        at_of[j][hh][nh][HD:HD + 1, :])
            rrow = p_dr.tile([1, 4 * 512], f32, tag="rrow", name="rrow")
            nc.sync.dma_start(out=rrow, in_=rr)
            # issue the 4 broadcast DMAs now; the muls are emitted later
            # (next iteration / tail) so the DVE FIFO never head-blocks.
            rms = []
            for hh in range(2):
                for nh in range(NH):
                    r = hh * NH + nh
                    rm = p_rm.tile([HD, 512], f32, tag="rm", name="rm")
                    seg = rrow[0:1, r * 512:(r + 1) * 512]
                    bcast = bass.AP(tensor=seg.tensor, offset=seg.offset,
                                    ap=[[0, HD]] + list(seg.ap[1:]))
                    nc.sync.dma_start(out=rm, in_=bcast)
                    rms.append(rm)
            at_of[j].append(rms)  # stash

        def emit_norm_mul(j, idx):
            # fused evacuation + normalization: attnT = psum * (1/denom)
            hh, nh = divmod(idx, NH)
            rm = at_of[j][2][idx]
            nc.vector.tensor_mul(
                attnT_sb[j][hh * HD:(hh + 1) * HD, nh * 512:(nh + 1) * 512],
                at_of[j][hh][nh][0:HD, :],
                rm)

        # ---- main pipelined loop ------------------------------------------
        # iteration j: scores/exp/mult for pair j; attention matmuls for
        # pair j-1 (k-steps fed from a flat queue, ~5 per slot from slot 1);
        # norm-muls for pair j-2 at slots 0..3; recip chain for pair j-1 at
        # the end.  Projections fill iteration 0's attention-free slots.
        for j in range(NP):
            Ws_of[j] = [[None] * TT, [None] * TT]
            if j - 2 in Ws_of:
                del Ws_of[j - 2]
            attn_q = []
            if j >= 1:
                alloc_at(j - 1)
                attn_q = list(range(TT))
            for i in range(TT):
                emit_scores(j, i)
                if j == 0:
                    for _ in range(2):
                        if proj_rest:
                            kind, a1, a2 = proj_rest.pop(0)
                            if kind == "q":
                                emit_qproj(a1, a2)
                            else:
                                emit_vproj(a1)
                else:
                    # pair j-2's norm muls first (they free the at psum ring
                    # the pair j-1 k-steps are about to WAR on)
                    if j >= 2 and i < 2:
                        emit_norm_mul(j - 2, 2 * i)
                        emit_norm_mul(j - 2, 2 * i + 1)
                    start = 2 if j >= 2 else 1
                    if i >= start:
                        take = 2 if i in (4, 5) else 1
                        for _ in range(take):
                            if attn_q:
                                emit_attn_kstep(j - 1, attn_q.pop(0))
            # drain leftover k-steps densely, then the recip chain
            for i in attn_q:
                emit_attn_kstep(j - 1, i)
            if j >= 1:
                emit_recip(j - 1)

        # ---- tail ----------------------------------------------------------
        # p3a = out-proj partial over k=0..2 (attnT of pairs 0..2).  The
        # k=0..1 steps of the first two partials fill the PE during the
        # pair-2 rm DMA latency; the k=2 steps wait on pair-2's norm muls
        # (emitted ahead of the partial evacs in the DVE FIFO -- no cycle).
        p3_part = [p_qv.tile([128, E], f32, tag=f"p3p{k}", name="p3p")
                   for k in range(TT)]
        p3_held = {}

        def p3a_start(mt):
            ps = ps_a.tile([128, 512], f32, tag="slot", name="pp")
            p3_held[mt] = ps
            for k in range(KT - 2):
                nc.tensor.matmul(
                    ps, attnT_sb[k][:, mt * 128:(mt + 1) * 128],
                    wo_sb[k], start=(k == 0), stop=False)

        def p3a_finish(mt):
            ps = p3_held.pop(mt)
            k = KT - 2
            nc.tensor.matmul(
                ps, attnT_sb[k][:, mt * 128:(mt + 1) * 128],
                wo_sb[k], start=False, stop=True)
            nc.vector.tensor_copy(p3_part[mt], ps)

        p3a_start(0)
        p3a_start(1)
        for idx in range(4):
            emit_norm_mul(2, idx)
        p3a_finish(0)
        p3a_finish(1)
        # pair 3 attention: at tiles come from ps_b (freed by norm(2) muls)
        alloc_at(NP - 1)
        for i in range(TT):
            emit_attn_kstep(NP - 1, i)
        emit_recip(NP - 1)
        for mt in range(2, TT):
            p3a_start(mt)
            p3a_finish(mt)
        for idx in range(4):
            emit_norm_mul(3, idx)

        # k=3 + bo2 bias, add the partial, store
        for mt in range(TT):
            ps = ps_a.tile([128, 512], f32, tag="slot", name="pp")
            nc.tensor.matmul(
                ps, attnT_sb[KT - 1][:, mt * 128:(mt + 1) * 128],
                wo_sb[KT - 1], start=True, stop=False)
            nc.tensor.matmul(ps, ones1, bo2_sb, start=False, stop=True)
            st = p_st.tile([128, E], f32, tag="st", name="st")
            nc.vector.tensor_add(st, ps, p3_part[mt])
            nc.sync.dma_start(out=out_d[mt * 128:(mt + 1) * 128, :], in_=st)

    nc.compile()
    return nc


def get_nc():
    if "nc" not in _cache:
        _cache["nc"] = _build_nc()
    return _cache["nc"]


def prep_inputs(query, pe, in_proj_weight, in_proj_bias, out_proj_weight,
                out_proj_bias):
    """Host-side sharding/layout prep. Returns per-core input maps."""
    query = np.asarray(query, dtype=np.float32)
    pe = np.asarray(pe, dtype=np.float32)
    in_proj_weight = np.asarray(in_proj_weight, dtype=np.float32)
    in_proj_bias = np.asarray(in_proj_bias, dtype=np.float32)
    out_proj_weight = np.asarray(out_proj_weight, dtype=np.float32)
    out_proj_bias = np.asarray(out_proj_bias, dtype=np.float32)

    wqT = np.ascontiguousarray(in_proj_weight[0:E].T).astype(np.float16)
    wvT = np.ascontiguousarray(in_proj_weight[2 * E:3 * E].T).astype(np.float16)
    woT = np.ascontiguousarray(out_proj_weight.T).astype(np.float16)
    bq = np.ascontiguousarray(in_proj_bias[0:E])
    bv = in_proj_bias[2 * E:3 * E]
    bo2 = (out_proj_weight @ bv + out_proj_bias).astype(np.float16)

    in_maps = []
    for b in range(N_CORES):
        xT = np.ascontiguousarray(query[:, b, :].T).astype(np.float16)
        peT = np.ascontiguousarray(pe[b].T).astype(np.float16)
        in_maps.append({
            "xT": xT, "peT": peT, "wqT": wqT, "wvT": wvT, "woT": woT,
            "bq": bq, "bo2": bo2, "ones1": np.ones(128, dtype=np.float16),
        })
    return in_maps


def kernel(query, pe, in_proj_weight, in_proj_bias, out_proj_weight,
           out_proj_bias):
    from concourse.bass_utils import run_bass_kernel_spmd

    nc = get_nc()
    in_maps = prep_inputs(query, pe, in_proj_weight, in_proj_bias,
                          out_proj_weight, out_proj_bias)
    res = run_bass_kernel_spmd(nc, in_maps, list(range(N_CORES)))
    out = np.empty((T, B, E), dtype=np.float32)
    for b in range(N_CORES):
        out[:, b, :] = res.results[b]["out"]
    return out
